# revision 1
# baseline (speedup 1.0000x reference)
"""Trainium2 Bass kernel for the SelfOrg spiking-network step.

Reference computation (per batch b, neuron n):
    z_out_new = BETA * z_out + z
    z_loo[b,j,n] = z_out_new[b, j + (j>=n)]            (leave-one-out gather)
    drive[b,n]  = sum_k x[b,k,n] * w[k,n]  (k < N_IN)
                + sum_j z_loo[b,j,n] * w[N_IN+j, n]
    v_new = ALPHA*v + drive - V_TH*z
    z_new = (v_new - V_TH > 0)

Strategy:
  * Batch-parallel over 8 cores (8 batches each).
  * The x-part is an elementwise-weighted reduction over k. Layout: k on
    SBUF partitions (p = k//16, s = k%16), n in the free dim. The vector
    engine does tmp = x*w in-place; the tensor engine reduces over
    partitions with a per-batch indicator stationary operand
    (lhsT[p, m] = (m==b)), accumulating all batches into one (8,512)
    PSUM tile with b on partitions.
  * The leave-one-out term is algebraically a dense matmul
    z_out_new @ Wf where Wf[m,n] = w[N_IN + m - (m>n), n], diag(Wf)=0.
    Wf is precomputed on the host; the (8,512)x(512,512) matmul runs on
    the tensor engine using 4 PE transposes of z_out_new as lhsT.

  Implementation notes:
  * Built with bacc.Bacc: TRN2 instructions have a single hardware
    sync-wait slot and Bacc's generate_event_semaphores pass splits
    multi-wait instructions (raw bass.Bass fails walrus codegen).
  * fp32 matmul streams at ~4 cycles/column (2 half-speed passes), so
    half the reduce slices are pre-folded on the (cheaper) vector
    engine before the PE reduce.
  * The HWDGE DMA ring is FIFO: small state tensors are queued first
    and the w chunks are interleaved with batch 0's x chunks so the
    first multiply starts ~10us in.
"""

import numpy as np

# model hyperparameters (must match the reference)
N_IN = 2048
NN = 512
BATCH = 64
DT, TAU_M, TAU_X = 0.05, 10.0, 2.0
ALPHA = 1.0 - DT / TAU_M
BETA = 1.0 - DT / TAU_X
V_TH = 2.0

NCORES = 8
BPC = BATCH // NCORES      # batches per core
P = 128                    # SBUF partitions
S = N_IN // P              # 16 k-rows folded per partition
FD = S * NN                # 8192 free elements of one batch tile
CHUNKS = 4                 # DMA / vector-multiply chunks per batch
CFD = FD // CHUNKS         # 2048 free elements per chunk
SPC = S // CHUNKS          # 4 reduce slices per chunk
XBUFS = 10                 # x chunk tiles in flight (DMA ahead of DVE)
TBUFS = 6                  # product chunk tiles in flight (DVE ahead of PE)


def _build_nc():
    import concourse.mybir as mybir
    from concourse import bacc
    from concourse.masks import make_identity
    from concourse.tile import TileContext

    f32 = mybir.dt.float32
    nc = bacc.Bacc("TRN2", name="selforg_step")

    x_h = nc.dram_tensor("x", [BPC, N_IN, NN], f32, kind="ExternalInput")
    v_h = nc.dram_tensor("v", [BPC, NN], f32, kind="ExternalInput")
    z_h = nc.dram_tensor("z", [BPC, NN], f32, kind="ExternalInput")
    zo_h = nc.dram_tensor("z_out", [BPC, NN], f32, kind="ExternalInput")
    w_h = nc.dram_tensor("w", [N_IN, NN], f32, kind="ExternalInput")
    wf_h = nc.dram_tensor("wf", [NN, NN], f32, kind="ExternalInput")
    out_h = nc.dram_tensor("out", [3, BPC, NN], f32, kind="ExternalOutput")

    # partition p <- x[b] bytes [32KB*p, 32KB*(p+1)): k = 16p + s
    x_r = x_h[:, :, :].rearrange("b (p s) n -> b p (s n)", p=P)
    w_r = w_h[:, :].rearrange("(p s) n -> p (s n)", p=P)
    wf_r = wf_h[:, :].rearrange("(t p) n -> p t n", p=P)

    with TileContext(nc) as tc:
        with (
            tc.tile_pool(name="const", bufs=1) as cpool,
            tc.tile_pool(name="xin", bufs=XBUFS) as xpool,
            tc.tile_pool(name="tmp", bufs=TBUFS) as tpool,
            tc.tile_pool(name="psum", bufs=1, space="PSUM") as ppool,
            tc.tile_pool(name="psum2", bufs=2, space="PSUM") as ppool2,
        ):
            # ---- input DMAs. The HWDGE ring is FIFO, so order = stream
            # order: tiny state tensors first, then w chunks interleaved
            # with batch 0's x chunks (the first multiply needs only w
            # chunk 0 + x chunk 0).
            v_sb = cpool.tile([BPC, NN], f32)
            z_sb = cpool.tile([BPC, NN], f32)
            zo_sb = cpool.tile([BPC, NN], f32)
            nc.sync.dma_start(v_sb[:, :], v_h[:, :])
            nc.sync.dma_start(z_sb[:, :], z_h[:, :])
            nc.sync.dma_start(zo_sb[:, :], zo_h[:, :])

            wf_sb = cpool.tile([P, 4 * NN], f32)
            nc.sync.dma_start(
                wf_sb[:, :].rearrange("p (t n) -> p t n", t=4), wf_r[:, :, :]
            )
            w_sb = cpool.tile([P, FD], f32)

            # per-batch indicator columns: ind[:, 8b + j] = (j == b)
            ind = cpool.tile([P, BPC * BPC], f32)
            nc.gpsimd.memset(ind[:, :], 0.0)
            for b in range(BPC):
                nc.gpsimd.memset(ind[:, 9 * b : 9 * b + 1], 1.0)

            ident = cpool.tile([BPC, BPC], f32)
            make_identity(nc, ident[:, :])

            # ---- lateral trace update ----
            zon_sb = cpool.tile([BPC, NN], f32)
            nc.vector.tensor_scalar_mul(zon_sb[:, :], zo_sb[:, :], BETA)
            nc.vector.tensor_add(zon_sb[:, :], zon_sb[:, :], z_sb[:, :])

            # transpose z_out_new: 4x (8,128) -> (128,8)
            zonT = cpool.tile([P, 4 * BPC], f32)
            for t in range(4):
                psum_t = ppool2.tile([P, BPC], f32, tag="tr")
                nc.tensor.transpose(
                    psum_t[:, :], zon_sb[:, t * P : (t + 1) * P], ident[:, :]
                )
                nc.vector.tensor_copy(zonT[:, t * BPC : (t + 1) * BPC], psum_t[:, :])

            # lateral drive: psum_lat[b,n] = sum_m zon[b,m] * Wf[m,n]
            lat_tile = ppool.tile([BPC, NN], f32, tag="lat")
            for t in range(4):
                nc.tensor.matmul(
                    lat_tile[:, :],
                    zonT[:, t * BPC : (t + 1) * BPC],
                    wf_sb[:, t * NN : (t + 1) * NN],
                    start=(t == 0),
                    stop=(t == 3),
                )

            # ---- main loop: drive[b,n] = sum_k x[b,k,n]*w[k,n] ----
            # Per (b, chunk): DMA x chunk -> DVE product -> PE indicator-
            # matmul reduce into psum_drive row b. The first FOLD_CHUNKS
            # chunks per batch get a half-width DVE fold (4 slices -> 2),
            # trading cheap DVE adds for expensive fp32 PE columns.
            def fold_this(b, c):
                return c < 2

            total_mms = sum(
                (SPC // 2 if fold_this(b, c) else SPC)
                for b in range(BPC) for c in range(CHUNKS)
            )
            psum_drive = ppool.tile([BPC, NN], f32, tag="drive")
            mm_idx = 0
            for b in range(BPC):
                for c in range(CHUNKS):
                    cs = slice(c * CFD, (c + 1) * CFD)
                    if b == 0:
                        # stream w chunk c just ahead of the x chunk using it
                        nc.sync.dma_start(w_sb[:, cs], w_r[:, cs])
                    xc = xpool.tile([P, CFD], f32, tag="xc")
                    nc.sync.dma_start(xc[:, :], x_r[b, :, cs])
                    tm = tpool.tile([P, CFD], f32, tag="tm")
                    nc.vector.tensor_mul(tm[:, :], xc[:, :], w_sb[:, cs])
                    if fold_this(b, c):
                        # fold slices (s0,s1) += (s2,s3)
                        nc.vector.tensor_add(
                            tm[:, : CFD // 2], tm[:, : CFD // 2], tm[:, CFD // 2 :]
                        )
                    for j in range(SPC // 2 if fold_this(b, c) else SPC):
                        nc.tensor.matmul(
                            psum_drive[:, :],
                            ind[:, BPC * b : BPC * (b + 1)],
                            tm[:, j * NN : (j + 1) * NN],
                            start=(mm_idx == 0),
                            stop=(mm_idx == total_mms - 1),
                        )
                        mm_idx += 1

            # ---- epilogue ----
            t1 = cpool.tile([BPC, NN], f32)
            nc.vector.tensor_scalar_mul(t1[:, :], v_sb[:, :], ALPHA)
            nc.vector.tensor_add(t1[:, :], t1[:, :], psum_drive[:, :])
            t2 = cpool.tile([BPC, NN], f32)
            nc.vector.tensor_scalar_mul(t2[:, :], z_sb[:, :], -V_TH)
            nc.vector.tensor_add(t2[:, :], t2[:, :], lat_tile[:, :])
            vn_sb = cpool.tile([BPC, NN], f32)
            nc.vector.tensor_add(vn_sb[:, :], t1[:, :], t2[:, :])

            zn_sb = cpool.tile([BPC, NN], f32)
            nc.vector.tensor_scalar(
                out=zn_sb[:, :],
                in0=vn_sb[:, :],
                scalar1=V_TH,
                scalar2=None,
                op0=mybir.AluOpType.is_gt,
            )

            nc.sync.dma_start(out_h[0, :, :], vn_sb[:, :])
            nc.sync.dma_start(out_h[1, :, :], zn_sb[:, :])
            nc.sync.dma_start(out_h[2, :, :], zon_sb[:, :])

    return nc


def _make_wf(w: np.ndarray) -> np.ndarray:
    """Wf[m,n] = w[N_IN + m - (m>n), n] off-diagonal, 0 on the diagonal."""
    wl = w[N_IN:]
    m = np.arange(NN)[:, None]
    n = np.arange(NN)[None, :]
    idx = np.minimum(np.where(m > n, m - 1, m), NN - 2)
    return np.where(m == n, np.float32(0.0), wl[idx, n]).astype(np.float32)


def _make_in_maps(x, v, z, z_out, w):
    w_x = np.ascontiguousarray(w[:N_IN], dtype=np.float32)
    wf = _make_wf(np.asarray(w, dtype=np.float32))
    in_maps = []
    for c in range(NCORES):
        sl = slice(c * BPC, (c + 1) * BPC)
        in_maps.append(
            {
                "x": np.ascontiguousarray(x[sl], dtype=np.float32),
                "v": np.ascontiguousarray(v[sl], dtype=np.float32),
                "z": np.ascontiguousarray(z[sl], dtype=np.float32),
                "z_out": np.ascontiguousarray(z_out[sl], dtype=np.float32),
                "w": w_x,
                "wf": wf,
            }
        )
    return in_maps


def run(x, v, z, z_out, w, trace=False):
    """Build + run on the 8 NeuronCores; returns (output, BassKernelResults)."""
    from concourse.bass_utils import run_bass_kernel_spmd

    nc = _build_nc()
    if not nc.is_finalized():
        nc.finalize()
    in_maps = _make_in_maps(x, v, z, z_out, w)
    res = run_bass_kernel_spmd(nc, in_maps, core_ids=list(range(NCORES)), trace=trace)
    full = np.concatenate([r["out"] for r in res.results], axis=1)
    return np.ascontiguousarray(full, dtype=np.float32), res


def kernel(x, v, z, z_out, w):
    out, _ = run(x, v, z, z_out, w)
    return out



# revision 5
# speedup vs baseline: 1.6049x; 1.6049x over previous
"""Trainium2 Bass kernel for the SelfOrg spiking-network step.

Reference computation (per batch b, neuron n):
    z_out_new = BETA * z_out + z
    z_loo[b,j,n] = z_out_new[b, j + (j>=n)]            (leave-one-out gather)
    drive[b,n]  = sum_k x[b,k,n] * w[k,n]  (k < N_IN)
                + sum_j z_loo[b,j,n] * w[N_IN+j, n]
    v_new = ALPHA*v + drive - V_TH*z
    z_new = (v_new - V_TH > 0)

Strategy:
  * Batch-parallel over 8 cores (8 batches each). The kernel is memory
    bound on streaming x; x and w are cast to fp16 on the host, which
    halves HBM traffic (error ~2^-11 per term, far inside the 2e-2
    tolerance) and unlocks the DVE 2x packed mode and the PE 16-bit
    1-column/cycle rate.
  * The x-part is an elementwise-weighted reduction over k. Layout: k on
    SBUF partitions (p = k//16, s = k%16), n in the free dim. The vector
    engine does tm = x*w; the tensor engine reduces over partitions with
    a per-batch indicator stationary operand (lhsT[p, m] = (m==b)),
    accumulating all batches into one (8,512) PSUM tile.
  * The leave-one-out term is algebraically a dense matmul
    z_out_new @ Wf where Wf[m,n] = w[N_IN + m - (m>n), n], diag(Wf)=0.
    Wf is precomputed on the host; the (8,512)x(512,512) matmul runs
    first in the same PSUM accumulation group as the x reduction.
  * Two HWDGE rings (sync + scalar/ACT) split the DMA issue stream:
    small state + w chunks on one ring, x chunks alternating, so
    descriptor issue (~0.6us each) never serializes the stream.
"""

import numpy as np

# model hyperparameters (must match the reference)
N_IN = 2048
NN = 512
BATCH = 64
DT, TAU_M, TAU_X = 0.05, 10.0, 2.0
ALPHA = 1.0 - DT / TAU_M
BETA = 1.0 - DT / TAU_X
V_TH = 2.0

NCORES = 8
BPC = BATCH // NCORES      # batches per core
P = 128                    # SBUF partitions
S = N_IN // P              # 16 k-rows folded per partition
FD = S * NN                # 8192 free elements of one batch tile
CHUNKS = 2                 # DMA / vector-multiply chunks per batch
CFD = FD // CHUNKS         # 4096 free elements per chunk
SPC = S // CHUNKS          # 8 reduce slices per chunk
XBUFS = 6                  # x chunk tiles in flight (DMA ahead of DVE)
TBUFS = 4                  # product chunk tiles in flight (DVE ahead of PE)


def _build_nc():
    import concourse.mybir as mybir
    from concourse import bacc
    from concourse.masks import make_identity
    from concourse.tile import TileContext

    f32 = mybir.dt.float32
    f16 = mybir.dt.float16
    nc = bacc.Bacc("TRN2", name="selforg_step")

    x_h = nc.dram_tensor("x", [BPC, N_IN, NN], f16, kind="ExternalInput")
    v_h = nc.dram_tensor("v", [BPC, NN], f32, kind="ExternalInput")
    z_h = nc.dram_tensor("z", [BPC, NN], f32, kind="ExternalInput")
    zo_h = nc.dram_tensor("z_out", [BPC, NN], f32, kind="ExternalInput")
    w_h = nc.dram_tensor("w", [N_IN, NN], f16, kind="ExternalInput")
    wf_h = nc.dram_tensor("wf", [NN, NN], f16, kind="ExternalInput")
    ind_h = nc.dram_tensor("ind", [P, BPC * BPC], f16, kind="ExternalInput")
    out_h = nc.dram_tensor("out", [BPC, 3, NN], f32, kind="ExternalOutput")

    # partition p <- x[b] bytes [16KB*p, 16KB*(p+1)): k = 16p + s
    x_r = x_h[:, :, :].rearrange("b (p s) n -> b p (s n)", p=P)
    w_r = w_h[:, :].rearrange("(p s) n -> p (s n)", p=P)
    wf_r = wf_h[:, :].rearrange("(t p) n -> p t n", p=P)
    # out[b, t, n] <- res[b, t*NN + n]
    out_r = out_h[:, :, :].rearrange("b t n -> b (t n)")

    with TileContext(nc) as tc:
        with (
            tc.tile_pool(name="const", bufs=1) as cpool,
            tc.tile_pool(name="xin", bufs=XBUFS) as xpool,
            tc.tile_pool(name="tmp", bufs=TBUFS) as tpool,
            tc.tile_pool(name="psum", bufs=1, space="PSUM") as ppool,
            tc.tile_pool(name="psum2", bufs=2, space="PSUM") as ppool2,
        ):
            # ---- input DMAs. Small state tensors first on the scalar
            # (ACT) HWDGE ring — they finish in ~1us and unblock the
            # lateral path; w chunks stream on the same ring just ahead
            # of the x chunks (sync ring) that consume them.
            v_sb = cpool.tile([BPC, NN], f32)
            z_sb = cpool.tile([BPC, NN], f32)
            zo_sb = cpool.tile([BPC, NN], f32)
            ind = cpool.tile([P, BPC * BPC], f16)
            wf_sb = cpool.tile([P, 4 * NN], f16)
            nc.scalar.dma_start(v_sb[:, :], v_h[:, :])
            nc.scalar.dma_start(z_sb[:, :], z_h[:, :])
            nc.scalar.dma_start(zo_sb[:, :], zo_h[:, :])
            nc.scalar.dma_start(ind[:, :], ind_h[:, :])
            nc.scalar.dma_start(
                wf_sb[:, :].rearrange("p (t n) -> p t n", t=4), wf_r[:, :, :]
            )
            w_sb = cpool.tile([P, FD], f16)

            ident = cpool.tile([BPC, BPC], f32)
            make_identity(nc, ident[:, :])

            # ---- output staging tile: [vn | zn | zon] in the free dim
            res = cpool.tile([BPC, 3 * NN], f32)
            vn = res[:, 0:NN]
            zn = res[:, NN : 2 * NN]
            zon = res[:, 2 * NN : 3 * NN]

            # ---- lateral trace update (fp32 output, fp16 copy for PE)
            nc.vector.tensor_scalar_mul(zon, zo_sb[:, :], BETA)
            nc.vector.tensor_add(zon, zon, z_sb[:, :])

            # av = ALPHA*v - V_TH*z, folded early so the epilogue is short
            av_sb = cpool.tile([BPC, NN], f32)
            nc.vector.tensor_scalar_mul(av_sb[:, :], z_sb[:, :], -V_TH)
            zv_sb = cpool.tile([BPC, NN], f32)
            nc.vector.tensor_scalar_mul(zv_sb[:, :], v_sb[:, :], ALPHA)
            nc.vector.tensor_add(av_sb[:, :], av_sb[:, :], zv_sb[:, :])

            # transpose z_out_new: 4x (8,128) -> (128,8), cast to fp16
            zonT = cpool.tile([P, 4 * BPC], f16)
            for t in range(4):
                psum_t = ppool2.tile([P, BPC], f32, tag="tr")
                nc.tensor.transpose(
                    psum_t[:, :], zon[:, t * P : (t + 1) * P], ident[:, :]
                )
                nc.vector.tensor_copy(zonT[:, t * BPC : (t + 1) * BPC], psum_t[:, :])

            # single PSUM accumulation group: 4 lateral matmuls first,
            # then the whole x reduction
            psum_drive = ppool.tile([BPC, NN], f32, tag="drive")
            total_mms = 4 + BPC * CHUNKS * SPC
            mm_idx = 0
            for t in range(4):
                nc.tensor.matmul(
                    psum_drive[:, :],
                    zonT[:, t * BPC : (t + 1) * BPC],
                    wf_sb[:, t * NN : (t + 1) * NN],
                    start=(mm_idx == 0),
                    stop=False,
                )
                mm_idx += 1

            # ---- main loop: drive[b,n] += sum_k x[b,k,n]*w[k,n] ----
            for b in range(BPC):
                for c in range(CHUNKS):
                    cs = slice(c * CFD, (c + 1) * CFD)
                    if b == 0:
                        # stream w chunk c just ahead of the x chunk using it
                        nc.scalar.dma_start(w_sb[:, cs], w_r[:, cs])
                    xc = xpool.tile([P, CFD], f16, tag="xc")
                    eng = nc.sync if (b * CHUNKS + c) % 2 == 0 else nc.scalar
                    eng.dma_start(xc[:, :], x_r[b, :, cs])
                    tm = tpool.tile([P, CFD], f16, tag="tm")
                    nc.vector.tensor_mul(tm[:, :], xc[:, :], w_sb[:, cs])
                    for j in range(SPC):
                        nc.tensor.matmul(
                            psum_drive[:, :],
                            ind[:, BPC * b : BPC * (b + 1)],
                            tm[:, j * NN : (j + 1) * NN],
                            start=False,
                            stop=(mm_idx == total_mms - 1),
                        )
                        mm_idx += 1

            # ---- epilogue ----
            nc.vector.tensor_add(vn, av_sb[:, :], psum_drive[:, :])
            nc.vector.tensor_scalar(
                out=zn,
                in0=vn,
                scalar1=V_TH,
                scalar2=None,
                op0=mybir.AluOpType.is_gt,
            )
            nc.sync.dma_start(out_r, res[:, :])

    return nc


def _make_wf(w: np.ndarray) -> np.ndarray:
    """Wf[m,n] = w[N_IN + m - (m>n), n] off-diagonal, 0 on the diagonal."""
    wl = w[N_IN:]
    m = np.arange(NN)[:, None]
    n = np.arange(NN)[None, :]
    idx = np.minimum(np.where(m > n, m - 1, m), NN - 2)
    return np.where(m == n, np.float32(0.0), wl[idx, n]).astype(np.float32)


def _make_ind() -> np.ndarray:
    """Indicator columns: ind[:, BPC*b + j] = (j == b)."""
    ind = np.zeros((P, BPC * BPC), dtype=np.float16)
    for b in range(BPC):
        ind[:, BPC * b + b] = 1.0
    return ind


def _make_in_maps(x, v, z, z_out, w):
    w16 = np.ascontiguousarray(w[:N_IN]).astype(np.float16)
    wf16 = _make_wf(np.asarray(w, dtype=np.float32)).astype(np.float16)
    ind = _make_ind()
    x16 = np.asarray(x).astype(np.float16)
    in_maps = []
    for c in range(NCORES):
        sl = slice(c * BPC, (c + 1) * BPC)
        in_maps.append(
            {
                "x": np.ascontiguousarray(x16[sl]),
                "v": np.ascontiguousarray(v[sl], dtype=np.float32),
                "z": np.ascontiguousarray(z[sl], dtype=np.float32),
                "z_out": np.ascontiguousarray(z_out[sl], dtype=np.float32),
                "w": w16,
                "wf": wf16,
                "ind": ind,
            }
        )
    return in_maps


def run(x, v, z, z_out, w, trace=False):
    """Build + run on the 8 NeuronCores; returns (output, BassKernelResults)."""
    from concourse.bass_utils import run_bass_kernel_spmd

    nc = _build_nc()
    if not nc.is_finalized():
        nc.finalize()
    in_maps = _make_in_maps(x, v, z, z_out, w)
    res = run_bass_kernel_spmd(nc, in_maps, core_ids=list(range(NCORES)), trace=trace)
    # per-core out is [BPC, 3, NN]; reassemble to [3, BATCH, NN]
    full = np.concatenate([r["out"].transpose(1, 0, 2) for r in res.results], axis=1)
    return np.ascontiguousarray(full, dtype=np.float32), res


def kernel(x, v, z, z_out, w):
    out, _ = run(x, v, z, z_out, w)
    return out


# revision 11
# speedup vs baseline: 1.6155x; 1.0066x over previous
"""Trainium2 Bass kernel for the SelfOrg spiking-network step.

Reference computation (per batch b, neuron n):
    z_out_new = BETA * z_out + z
    z_loo[b,j,n] = z_out_new[b, j + (j>=n)]            (leave-one-out gather)
    drive[b,n]  = sum_k x[b,k,n] * w[k,n]  (k < N_IN)
                + sum_j z_loo[b,j,n] * w[N_IN+j, n]
    v_new = ALPHA*v + drive - V_TH*z
    z_new = (v_new - V_TH > 0)

Strategy:
  * Batch-parallel over 8 cores (8 batches each). The kernel is memory
    bound on streaming x; x and w are cast to fp16 on the host, which
    halves HBM traffic (error ~2^-11 per term, far inside the 2e-2
    tolerance) and unlocks the DVE 2x packed mode and the PE 16-bit
    1-column/cycle rate.
  * The x-part is an elementwise-weighted reduction over k. Layout: k on
    SBUF partitions (p = k//16, s = k%16), n in the free dim. The vector
    engine does tm = x*w; the tensor engine reduces over partitions with
    a per-batch indicator stationary operand (lhsT[p, m] = (m==b)),
    accumulating all batches into one (8,512) PSUM tile.
  * The leave-one-out term is algebraically a dense matmul
    z_out_new @ Wf where Wf[m,n] = w[N_IN + m - (m>n), n], diag(Wf)=0.
    Wf is precomputed on the host; the (8,512)x(512,512) matmul runs
    first in the same PSUM accumulation group as the x reduction.
  * Two HWDGE rings (sync + scalar/ACT) split the DMA issue stream:
    small state + w chunks on one ring, x chunks alternating, so
    descriptor issue (~0.6us each) never serializes the stream.
"""

import numpy as np

# model hyperparameters (must match the reference)
N_IN = 2048
NN = 512
BATCH = 64
DT, TAU_M, TAU_X = 0.05, 10.0, 2.0
ALPHA = 1.0 - DT / TAU_M
BETA = 1.0 - DT / TAU_X
V_TH = 2.0

NCORES = 8
BPC = BATCH // NCORES      # batches per core
P = 128                    # SBUF partitions
S = N_IN // P              # 16 k-rows folded per partition
FD = S * NN                # 8192 free elements of one batch tile
CHUNKS = 4                 # DMA / vector-multiply chunks per batch
CFD = FD // CHUNKS         # 2048 free elements per chunk
SPC = S // CHUNKS          # 4 reduce slices per chunk
XBUFS = 8                  # x chunk tiles in flight (DMA ahead of DVE)
TBUFS = 6                  # product chunk tiles in flight (DVE ahead of PE)


def _build_nc():
    import concourse.mybir as mybir
    from concourse import bacc
    from concourse.masks import make_identity
    from concourse.tile import TileContext

    f32 = mybir.dt.float32
    f16 = mybir.dt.float16
    nc = bacc.Bacc("TRN2", name="selforg_step")

    x_h = nc.dram_tensor("x", [BPC, N_IN, NN], f16, kind="ExternalInput")
    v_h = nc.dram_tensor("v", [BPC, NN], f32, kind="ExternalInput")
    z_h = nc.dram_tensor("z", [BPC, NN], f32, kind="ExternalInput")
    zo_h = nc.dram_tensor("z_out", [BPC, NN], f32, kind="ExternalInput")
    w_h = nc.dram_tensor("w", [N_IN, NN], f16, kind="ExternalInput")
    wf_h = nc.dram_tensor("wf", [NN, NN], f16, kind="ExternalInput")
    ind_h = nc.dram_tensor("ind", [P, BPC * BPC], f16, kind="ExternalInput")
    out_h = nc.dram_tensor("out", [BPC, 3, NN], f32, kind="ExternalOutput")

    # partition p <- x[b] bytes [16KB*p, 16KB*(p+1)): k = 16p + s
    x_r = x_h[:, :, :].rearrange("b (p s) n -> b p (s n)", p=P)
    w_r = w_h[:, :].rearrange("(p s) n -> p (s n)", p=P)
    wf_r = wf_h[:, :].rearrange("(t p) n -> p t n", p=P)
    # out[b, t, n] <- res[b, t*NN + n]
    out_r = out_h[:, :, :].rearrange("b t n -> b (t n)")

    with TileContext(nc) as tc:
        with (
            tc.tile_pool(name="const", bufs=1) as cpool,
            tc.tile_pool(name="xin", bufs=XBUFS) as xpool,
            tc.tile_pool(name="tmp", bufs=TBUFS) as tpool,
            tc.tile_pool(name="psum", bufs=1, space="PSUM") as ppool,
            tc.tile_pool(name="psum2", bufs=2, space="PSUM") as ppool2,
            tc.tile_pool(name="psum3", bufs=1, space="PSUM") as ppool3,
        ):
            # ---- input DMAs. Everything streams on the single sync
            # HWDGE ring in exact need-order (one ring already saturates
            # the 16 SDMA engines at ~358 GB/s; a second ring only
            # fair-shares the same engines and delays the critical first
            # chunks). Only the output DMA uses the scalar ring.
            v_sb = cpool.tile([BPC, NN], f32)
            z_sb = cpool.tile([BPC, NN], f32)
            zo_sb = cpool.tile([BPC, NN], f32)
            ind = cpool.tile([P, BPC * BPC], f16)
            wf_sb = cpool.tile([P, 4 * NN], f16)
            nc.sync.dma_start(zo_sb[:, :], zo_h[:, :])
            nc.sync.dma_start(z_sb[:, :], z_h[:, :])
            nc.sync.dma_start(v_sb[:, :], v_h[:, :])
            nc.sync.dma_start(ind[:, :], ind_h[:, :])
            w_sb = cpool.tile([P, FD], f16)

            ident = cpool.tile([BPC, BPC], f32)
            make_identity(nc, ident[:, :])

            # ---- output staging tile: [vn | zn | zon] in the free dim
            res = cpool.tile([BPC, 3 * NN], f32)
            vn = res[:, 0:NN]
            zn = res[:, NN : 2 * NN]
            zon = res[:, 2 * NN : 3 * NN]

            # ---- lateral trace update (fp32 output, fp16 copy for PE)
            nc.vector.tensor_scalar_mul(zon, zo_sb[:, :], BETA)
            nc.vector.tensor_add(zon, zon, z_sb[:, :])

            # av = ALPHA*v - V_TH*z, folded early so the epilogue is short
            av_sb = cpool.tile([BPC, NN], f32)
            nc.vector.tensor_scalar_mul(av_sb[:, :], z_sb[:, :], -V_TH)
            zv_sb = cpool.tile([BPC, NN], f32)
            nc.vector.tensor_scalar_mul(zv_sb[:, :], v_sb[:, :], ALPHA)
            nc.vector.tensor_add(av_sb[:, :], av_sb[:, :], zv_sb[:, :])

            # transpose z_out_new early (PE is idle during the DMA ramp):
            # 4x (8,128) -> (128,8), cast to fp16
            zonT = cpool.tile([P, 4 * BPC], f16)
            for t in range(4):
                psum_t = ppool2.tile([P, BPC], f32, tag="tr")
                nc.tensor.transpose(
                    psum_t[:, :], zon[:, t * P : (t + 1) * P], ident[:, :]
                )
                nc.vector.tensor_copy(zonT[:, t * BPC : (t + 1) * BPC], psum_t[:, :])

            # ---- main loop: drive[b,n] = sum_k x[b,k,n]*w[k,n] ----
            psum_drive = ppool.tile([BPC, NN], f32, tag="drive")
            total_mms = BPC * CHUNKS * SPC
            mm_idx = 0
            for b in range(BPC):
                for c in range(CHUNKS):
                    cs = slice(c * CFD, (c + 1) * CFD)
                    if b == 0:
                        # stream w chunk c just ahead of the x chunk using it
                        nc.sync.dma_start(w_sb[:, cs], w_r[:, cs])
                    xc = xpool.tile([P, CFD], f16, tag="xc")
                    nc.sync.dma_start(xc[:, :], x_r[b, :, cs])
                    if b == 0 and c == CHUNKS - 1:
                        # wf rides mid-stream; only needed by the tail mms
                        nc.sync.dma_start(
                            wf_sb[:, :].rearrange("p (t n) -> p t n", t=4),
                            wf_r[:, :, :],
                        )
                    tm = tpool.tile([P, CFD], f16, tag="tm")
                    nc.vector.tensor_mul(tm[:, :], xc[:, :], w_sb[:, cs])
                    for j in range(SPC):
                        nc.tensor.matmul(
                            psum_drive[:, :],
                            ind[:, BPC * b : BPC * (b + 1)],
                            tm[:, j * NN : (j + 1) * NN],
                            start=(mm_idx == 0),
                            stop=(mm_idx == total_mms - 1),
                        )
                        mm_idx += 1

            # lateral drive: lat[b,n] = sum_m zon[b,m] * Wf[m,n]
            lat_tile = ppool3.tile([BPC, NN], f32, tag="lat")
            for t in range(4):
                nc.tensor.matmul(
                    lat_tile[:, :],
                    zonT[:, t * BPC : (t + 1) * BPC],
                    wf_sb[:, t * NN : (t + 1) * NN],
                    start=(t == 0),
                    stop=(t == 3),
                )

            # ---- epilogue ----
            nc.vector.tensor_add(vn, av_sb[:, :], psum_drive[:, :])
            nc.vector.tensor_add(vn, vn, lat_tile[:, :])
            nc.vector.tensor_scalar(
                out=zn,
                in0=vn,
                scalar1=V_TH,
                scalar2=None,
                op0=mybir.AluOpType.is_gt,
            )
            nc.scalar.dma_start(out_r, res[:, :])

    return nc


def _make_wf(w: np.ndarray) -> np.ndarray:
    """Wf[m,n] = w[N_IN + m - (m>n), n] off-diagonal, 0 on the diagonal."""
    wl = w[N_IN:]
    m = np.arange(NN)[:, None]
    n = np.arange(NN)[None, :]
    idx = np.minimum(np.where(m > n, m - 1, m), NN - 2)
    return np.where(m == n, np.float32(0.0), wl[idx, n]).astype(np.float32)


def _make_ind() -> np.ndarray:
    """Indicator columns: ind[:, BPC*b + j] = (j == b)."""
    ind = np.zeros((P, BPC * BPC), dtype=np.float16)
    for b in range(BPC):
        ind[:, BPC * b + b] = 1.0
    return ind


def _make_in_maps(x, v, z, z_out, w):
    w16 = np.ascontiguousarray(w[:N_IN]).astype(np.float16)
    wf16 = _make_wf(np.asarray(w, dtype=np.float32)).astype(np.float16)
    ind = _make_ind()
    x16 = np.asarray(x).astype(np.float16)
    in_maps = []
    for c in range(NCORES):
        sl = slice(c * BPC, (c + 1) * BPC)
        in_maps.append(
            {
                "x": np.ascontiguousarray(x16[sl]),
                "v": np.ascontiguousarray(v[sl], dtype=np.float32),
                "z": np.ascontiguousarray(z[sl], dtype=np.float32),
                "z_out": np.ascontiguousarray(z_out[sl], dtype=np.float32),
                "w": w16,
                "wf": wf16,
                "ind": ind,
            }
        )
    return in_maps


def run(x, v, z, z_out, w, trace=False):
    """Build + run on the 8 NeuronCores; returns (output, BassKernelResults)."""
    from concourse.bass_utils import run_bass_kernel_spmd

    nc = _build_nc()
    if not nc.is_finalized():
        nc.finalize()
    in_maps = _make_in_maps(x, v, z, z_out, w)
    res = run_bass_kernel_spmd(nc, in_maps, core_ids=list(range(NCORES)), trace=trace)
    # per-core out is [BPC, 3, NN]; reassemble to [3, BATCH, NN]
    full = np.concatenate([r["out"].transpose(1, 0, 2) for r in res.results], axis=1)
    return np.ascontiguousarray(full, dtype=np.float32), res


def kernel(x, v, z, z_out, w):
    out, _ = run(x, v, z, z_out, w)
    return out
